# revision 38
# baseline (speedup 1.0000x reference)
"""Multi-head causal self-attention (B=2, S=2048, D=1024, H=16) on 8 TRN2 cores.

Sharding: head-parallel for QKV+attention (core c owns heads {2c, 2c+1}),
token-parallel for the output projection (core c owns tokens
[256c, 256c+256) of each batch), bridged by one AllToAll of the
normalized context per batch — 8x less wire traffic than gathering or
reduce-scattering partial outputs, since nothing is replicated.

The PE clock gate (HAM) re-throttles to 1.2GHz whenever a ~3.4us window
is fully idle, and only recovers on a fully-busy window; a single
attention stream's PE->ACT(exp)->DVE(mask)->PE chain stalls the PE every
couple of kg steps. So the two batches' attention streams are ZIPPED
kg-by-kg with a one-chunk lag (batch 1 trails): each stream's exp
latency is hidden behind the other stream's matmuls, the PE queue never
drains, and batch 0's A2A still fires a chunk before batch 1 finishes.
bc/normalize consumers are emitted one h-slot late so the PE queue never
waits on the l-reciprocal chain; batch 0's output projection fills batch
1's final solo chunk; batch 1's output projection is the only tail.

Per core (matmul operands bf16; PSUM accumulation f32):
  stage B: Q^T/K^T/V^T = (x @ W{q,k,v}[:, c-slice] + b)^T   [128, 4096]
  stage C: V^T -> V_aug [tok, 65] tiles (col 64 = ones -> l row)
  stage D: scores^T = K^T.T @ Q^T tiles (PE), exp (ACT, scale=1/8,
           diagonal blocks packed so no masked column is exp'd),
           post-exp multiplicative causal mask (DVE), ctx^T accum (PE)
  stage E: l row -> r = exp(-ln(l)) (ACT, same table as exp), PE
           outer-product broadcast, normalize -> bf16
  stage F: per batch: AllToAll ctx, then out = Wo^T-tiles @ ctx_full
           + bo for this core's 256 tokens, full Wo

Host: x pre-transposed bf16; weights host-pre-tiled to [p, kt, n] so all
loads are contiguous; output reassembled token-wise from yT2.
"""

import sys

for p in ("/opt/trn_rl_repo", "/root/.axon_site/_ro/trn_rl_repo"):
    if p not in sys.path:
        sys.path.insert(0, p)

from collections import deque

import numpy as np

import bass_rust
import concourse.bass as bass
import concourse.mybir as mybir
from concourse.bass_utils import run_bass_kernel_spmd
from concourse.masks import make_identity
from concourse.tile import TileContext

B, S, D = 2, 2048, 1024
H, DH = 16, 64
T = B * S              # 4096 tokens
NC = 8                 # cores
HG = D // NC           # 128 qkv dims per core (2 heads)
KT_D = D // 128        # 8 contraction tiles over d_model
QC = 512               # q-chunk width
NQC = S // QC          # 4 q-chunks per batch
TPC = S // NC          # 256 tokens per core per batch (out-proj sharding)
INV_SCALE = 1.0 / float(np.sqrt(DH))  # 1/8
F32 = mybir.dt.float32
F32R = mybir.dt.float32r
BF16 = mybir.dt.bfloat16


def _split_waits(nc, max_waits=1):
    """This walrus build accepts one sync-wait per instruction; Tile sometimes
    emits more. Split extras into preceding NoOps on the same engine."""
    n = 0
    for f in nc.m.functions:
        for bb in f.blocks:
            out = []
            for inst in bb.instructions:
                si = getattr(inst, "sync_info", None)
                if si is not None and si.on_wait and len(si.on_wait) > max_waits:
                    waits = list(si.on_wait)
                    head, rest = waits[:-max_waits], waits[-max_waits:]
                    k = 0
                    while head:
                        chunk, head = head[:max_waits], head[max_waits:]
                        out.append(mybir.InstNoOp(
                            name=f"{inst.name}-wsplit-{k}", ins=[], outs=[],
                            engine=inst.engine,
                            sync_info=bass_rust.SyncInfo(on_wait=chunk, on_update=[]),
                        ))
                        k += 1
                    si.on_wait = rest
                    n += 1
                out.append(inst)
            bb.instructions = out
    return n


def build_module():
    nc = bass.Bass()

    # weights arrive host-pre-tiled ([p, kt, n] flattened) so the loads are
    # fully contiguous DMAs instead of 256B-descriptor gather patterns
    xT = nc.dram_tensor("xT", [D, T], BF16, kind="ExternalInput")
    wq = nc.dram_tensor("wq", [128, KT_D * HG], BF16, kind="ExternalInput")
    wk = nc.dram_tensor("wk", [128, KT_D * HG], BF16, kind="ExternalInput")
    wv = nc.dram_tensor("wv", [128, KT_D * HG], BF16, kind="ExternalInput")
    wo = nc.dram_tensor("wo", [128, KT_D * D], BF16, kind="ExternalInput")
    bq = nc.dram_tensor("bq", [HG, 1], F32, kind="ExternalInput")
    bk = nc.dram_tensor("bk", [HG, 1], F32, kind="ExternalInput")
    bv = nc.dram_tensor("bv", [HG, 1], F32, kind="ExternalInput")
    bo = nc.dram_tensor("bo", [128, KT_D], F32, kind="ExternalInput")
    # output: this core's TPC tokens of each batch, all D dims
    yT2 = nc.dram_tensor("yT2", [D, B * TPC], F32, kind="ExternalOutput")

    # AllToAll buffers per batch: [shard/rank, 128, TPC]
    a2a_in = [nc.dram_tensor(f"a2i{b}", [NC, HG, TPC], BF16) for b in range(B)]
    a2a_out = [nc.dram_tensor(f"a2o{b}", [NC, HG, TPC], BF16) for b in range(B)]

    with TileContext(nc) as tc:
        with tc.tile_pool(name="persist", bufs=1) as pp:
            w_sb = {}
            for name, dram in (("wq", wq), ("wk", wk), ("wv", wv)):
                t = pp.tile([128, KT_D, HG], BF16, name=f"{name}_sb", tag=f"{name}_sb")
                nc.sync.dma_start(out=t[:], in_=dram[:].rearrange("p (kt n) -> p kt n", n=HG))
                w_sb[name] = t
            b_sb = {}
            for name, dram in (("bq", bq), ("bk", bk), ("bv", bv)):
                t = pp.tile([HG, 1], F32, name=f"{name}_sb", tag=f"{name}_sb")
                nc.sync.dma_start(out=t[:], in_=dram[:])
                b_sb[name] = t
            # Wo/bo tiles allocated now, loaded at D-phase start: the 2MB
            # load must not compete with stage B's x/w streaming
            wo_sb = pp.tile([128, KT_D, D], BF16, name="wo_sb", tag="wo_sb")
            bo_sb = pp.tile([128, KT_D], F32, name="bo_sb", tag="bo_sb")

            ident_f = pp.tile([128, 128], F32, name="ident_f", tag="ident_f")
            make_identity(nc, ident_f[:])
            ident = pp.tile([128, 128], BF16, name="ident", tag="ident")
            nc.vector.tensor_copy(ident[:], ident_f[:])
            # multiplicative causal mask for a diagonal 128x128 tile of
            # scores^T: keep [r, c] where r <= c (k <= q)
            tri_f = pp.tile([128, 128], F32, name="tri_f", tag="tri_f")
            nc.gpsimd.memset(tri_f[:], 1.0)
            nc.gpsimd.affine_select(
                out=tri_f[:], in_=tri_f[:],
                compare_op=mybir.AluOpType.is_ge, fill=0.0,
                base=0, pattern=[[1, 128]], channel_multiplier=-1,
            )
            tri01 = pp.tile([128, 128], BF16, name="tri01", tag="tri01")
            nc.vector.tensor_copy(tri01[:], tri_f[:])
            # ones row at partition 64 (base partition of the l row); bf16 so
            # the bc matmul's weight load uses the fast path
            ones_f = pp.tile([65, DH], F32, name="ones_f", tag="ones_f")
            nc.vector.memset(ones_f[:], 1.0)
            ones_r = pp.tile([65, DH], BF16, name="ones_r", tag="ones_r")
            nc.vector.tensor_copy(ones_r[:], ones_f[:])
            ones128 = pp.tile([128, B * 2 * (S // 128)], F32, name="ones128",
                              tag="ones128")
            nc.vector.memset(ones128[:], 1.0)

            qkvT = {}
            for name in ("qT", "kT", "vT"):
                qkvT[name] = [pp.tile([128, S], BF16, name=f"{name}{b}", tag=f"{name}{b}")
                              for b in range(B)]

            vaug = pp.tile([128, B * 2, S // 128, DH + 1], BF16, name="vaug", tag="vaug")
            nc.vector.tensor_copy(vaug[:, :, :, DH:DH + 1], ones128[:, :])
            # [65 used partitions, pair, q]; row 64 = l
            ctxu = pp.tile([128, B * 2, S], F32, name="ctxu", tag="ctxu")

            # ---------------- stage B+C, both batches ----------------
            # 3 accumulator tags x 2 bufs: adjacent 512-token chunks
            # accumulate in parallel banks, so chunk boundaries never wait
            # on the bias-add drains
            with (
                tc.tile_pool(name="xt0_pool", bufs=5) as xt0_pool,
                tc.tile_pool(name="psB", bufs=2, space="PSUM") as psB_pool,
                tc.tile_pool(name="psT", bufs=2, space="PSUM") as psT_pool,
            ):
                for b in range(B):
                    for tqg in range(2):
                        t0 = b * S + tqg * 1024
                        acc = {}
                        for ch in range(2):
                            for wname in ("wq", "wk", "wv"):
                                acc[(ch, wname)] = psB_pool.tile(
                                    [128, 512], F32, name=f"ps{wname}",
                                    tag=f"ps{wname}")
                        for kt in range(KT_D):
                            xt = xt0_pool.tile([128, 1024], BF16, name="xt", tag="xt")
                            nc.sync.dma_start(
                                out=xt[:],
                                in_=xT[kt * 128:(kt + 1) * 128, t0:t0 + 1024])
                            for ch in range(2):
                                for wname in ("wq", "wk", "wv"):
                                    nc.tensor.matmul(
                                        acc[(ch, wname)][:],
                                        w_sb[wname][:, kt, :],
                                        xt[:, ch * 512:(ch + 1) * 512],
                                        start=(kt == 0), stop=(kt == KT_D - 1),
                                    )
                        for ch in range(2):
                            for wname, bname in (("wq", "bq"), ("wk", "bk"),
                                                 ("wv", "bv")):
                                nc.vector.tensor_scalar_add(
                                    out=qkvT[wname.replace("w", "") + "T"][b][
                                        :, tqg * 1024 + ch * 512:
                                        tqg * 1024 + (ch + 1) * 512],
                                    in0=acc[(ch, wname)][:],
                                    scalar1=b_sb[bname][:, 0:1],
                                )
                    # ---- stage C for batch b ----
                    for h in range(2):
                        pr = b * 2 + h
                        for g in range(2):
                            pst = psT_pool.tile([128, 512], BF16, name="pst",
                                                tag="pst")
                            for j in range(8):
                                kt = g * 8 + j
                                nc.tensor.transpose(
                                    out=pst[:, j * DH:(j + 1) * DH],
                                    in_=qkvT["vT"][b][h * DH:(h + 1) * DH,
                                                      kt * 128:(kt + 1) * 128],
                                    identity=ident[h * DH:(h + 1) * DH,
                                                   h * DH:(h + 1) * DH],
                                )
                            nc.vector.tensor_copy(
                                vaug[:, pr, g * 8:(g + 1) * 8, 0:DH],
                                pst[:],
                            )

            # ------- stages D-F: zipped attention streams -------
            with (
                tc.tile_pool(name="psS", bufs=2, space="PSUM") as psS_pool,
                tc.tile_pool(name="psC", bufs=2, space="PSUM") as psC_pool,
                tc.tile_pool(name="mps", bufs=2, space="PSUM") as mps_pool,
                tc.tile_pool(name="exp_pool", bufs=4) as exp_pool,
                tc.tile_pool(name="rpool", bufs=4) as rpool,
                tc.tile_pool(name="cn_pool", bufs=4) as cn_pool,
                tc.tile_pool(name="cf_pool", bufs=2) as cf_pool,
                tc.tile_pool(name="yo_pool", bufs=2) as yo_pool,
            ):
                nc.gpsimd.dma_start(
                    out=wo_sb[:], in_=wo[:].rearrange("p (kt n) -> p kt n", n=D))
                nc.gpsimd.dma_start(out=bo_sb[:], in_=bo[:])

                r_tiles = {}
                cn_tiles = {}
                ctxf_tiles = {}
                pending = []

                def gen_D(b, qc, h):
                    # attention for (b, qc, h) as a generator: yields after
                    # each kg step so two streams can interleave in the PE
                    # queue; ends with the ctx copy + r chain (ACT/DVE only)
                    q0 = qc * QC
                    n_kt = q0 // 128 + 4
                    pr = b * 2 + h
                    qT_h = qkvT["qT"][b][h * DH:(h + 1) * DH, :]
                    kT_h = qkvT["kT"][b][h * DH:(h + 1) * DH, :]
                    ps_ctx = psC_pool.tile([128, QC], F32, name="ps_ctx",
                                           tag="ps_ctx")

                    def ctx_mms(ka, kb, offa, offb, ex):
                        nc.tensor.matmul(
                            ps_ctx[0:DH + 1, offa:512],
                            vaug[:, pr, ka, :],
                            ex[:, offa:512],
                            start=(ka == 0), stop=False,
                            skip_group_check=True,
                        )
                        nc.tensor.matmul(
                            ps_ctx[0:DH + 1, offb:512],
                            vaug[:, pr, kb, :],
                            ex[:, 512:1024 - offb],
                            start=False, stop=(kb == n_kt - 1),
                            skip_group_check=True,
                        )

                    prev_ctx = None
                    for kg in range(n_kt // 2):
                        ka, kb = 2 * kg, 2 * kg + 1
                        offa = max(0, ka * 128 - q0)
                        offb = max(0, kb * 128 - q0)
                        ps_s = psS_pool.tile([128, 1024], F32, name="ps_s",
                                             tag="ps_s")
                        # kb's block packed at column 512 (width 512-offb):
                        # the exp range [offa:1024-offb] has no dead gap
                        nc.tensor.matmul(
                            ps_s[:, offa:512],
                            kT_h[:, ka * 128:(ka + 1) * 128],
                            qT_h[:, q0 + offa:q0 + 512],
                            start=True, stop=True,
                        )
                        nc.tensor.matmul(
                            ps_s[:, 512:1024 - offb],
                            kT_h[:, kb * 128:(kb + 1) * 128],
                            qT_h[:, q0 + offb:q0 + 512],
                            start=True, stop=True,
                        )
                        # the PREVIOUS kg's ctx matmuls go behind this kg's
                        # scores: their exp input is ready by now, so the PE
                        # queue never waits on the exp chain
                        if prev_ctx is not None:
                            prev_ctx()
                        ex = exp_pool.tile([128, 1024], BF16, name="ex", tag="ex")
                        nc.scalar.activation(
                            out=ex[:, offa:1024 - offb],
                            in_=ps_s[:, offa:1024 - offb],
                            func=mybir.ActivationFunctionType.Exp,
                            scale=INV_SCALE,
                        )
                        if ka * 128 >= q0:
                            nc.vector.tensor_mul(
                                out=ex[:, offa:offa + 128],
                                in0=ex[:, offa:offa + 128],
                                in1=tri01[:],
                            )
                        if kb * 128 >= q0:
                            nc.vector.tensor_mul(
                                out=ex[:, 512:640],
                                in0=ex[:, 512:640],
                                in1=tri01[:],
                            )
                        prev_ctx = (lambda a=ka, b2=kb, oa=offa, ob=offb,
                                    e=ex: ctx_mms(a, b2, oa, ob, e))
                        yield
                    prev_ctx()
                    nc.vector.tensor_copy(
                        ctxu[0:DH + 1, pr, q0:q0 + 512],
                        ps_ctx[0:DH + 1, :],
                    )
                    # r = 1/l = exp(-ln(l)): ln/exp share the attention exp's
                    # ACT table (Reciprocal would force table reloads)
                    ln_f = rpool.tile([65, QC], F32, name="ln_f", tag="ln_f")
                    nc.scalar.activation(
                        out=ln_f[64:65, :], in_=ctxu[64:65, pr, q0:q0 + QC],
                        func=mybir.ActivationFunctionType.Ln)
                    r_t = rpool.tile([65, QC], BF16, name="r_t", tag="r_t")
                    nc.scalar.activation(
                        out=r_t[64:65, :], in_=ln_f[64:65, :],
                        func=mybir.ActivationFunctionType.Exp, scale=-1.0)
                    r_tiles[(b, qc, h)] = r_t

                def drive(gens, filler=None, fe=3):
                    gens = [g for g in gens if g is not None]
                    k = 0
                    while gens:
                        for g in list(gens):
                            try:
                                next(g)
                            except StopIteration:
                                gens.remove(g)
                        k += 1
                        if filler and k % fe == 0:
                            filler.popleft()()

                def emit_bcnorm(b, qc, h):
                    # deferred one h-slot: the PE bc matmul's reciprocal input
                    # is long since ready, so the PE queue never stalls here
                    q0 = qc * QC
                    pr = b * 2 + h
                    if h == 0:
                        cn_tiles[(b, qc)] = cn_pool.tile(
                            [128, QC], BF16, name="cn", tag="cn")
                    cn = cn_tiles[(b, qc)]
                    r_t = r_tiles.pop((b, qc, h))
                    bc = mps_pool.tile([128, QC], F32, name="bc", tag="mps")
                    nc.tensor.matmul(
                        bc[0:DH, :],
                        ones_r[64:65, 0:DH],
                        r_t[64:65, :],
                        start=True, stop=True,
                    )
                    nc.vector.tensor_mul(
                        out=cn[h * DH:(h + 1) * DH, :],
                        in0=ctxu[0:DH, pr, q0:q0 + QC],
                        in1=bc[0:DH, :],
                    )
                    if h == 1:
                        # chunk done: ship to the A2A input buffer (token
                        # groups 2qc, 2qc+1). Sync queue — the gpsimd queue
                        # blocks on in-flight collectives.
                        nc.sync.dma_start(
                            out=a2a_in[b][:].rearrange("g p n -> p g n")[
                                :, 2 * qc:2 * qc + 2, :],
                            in_=cn[:].rearrange("p (g n) -> p g n", g=2),
                        )
                        del cn_tiles[(b, qc)]

                def emit_a2a(b):
                    nc.gpsimd.collective_compute(
                        "AllToAll",
                        mybir.AluOpType.bypass,
                        ins=[a2a_in[b][:]],
                        outs=[a2a_out[b][:]],
                        replica_groups=[list(range(NC))],
                    )
                    ctxf = cf_pool.tile([128, KT_D, TPC], BF16, name="ctxf",
                                        tag="ctxf")
                    nc.gpsimd.dma_start(
                        out=ctxf[:],
                        in_=a2a_out[b][:].rearrange("kt p n -> p kt n"))
                    ctxf_tiles[b] = ctxf

                def u_outproj(b, og):
                    # two out-dim tiles of batch b's token-sharded output
                    # projection (shares the mps bank with bc)
                    def f():
                        ctxf = ctxf_tiles[b]
                        ps_o = mps_pool.tile([128, QC], F32, name="ps_o",
                                             tag="mps")
                        for sub in range(2):
                            ot = og * 2 + sub
                            for kt in range(KT_D):
                                nc.tensor.matmul(
                                    ps_o[:, sub * TPC:(sub + 1) * TPC],
                                    wo_sb[:, kt, ot * 128:(ot + 1) * 128],
                                    ctxf[:, kt, :],
                                    start=(kt == 0), stop=(kt == KT_D - 1),
                                    skip_group_check=True,
                                )
                        yo = yo_pool.tile([128, 2, TPC], F32, name="yo", tag="yo")
                        for sub in range(2):
                            ot = og * 2 + sub
                            nc.vector.tensor_scalar_add(
                                out=yo[:, sub, :],
                                in0=ps_o[:, sub * TPC:(sub + 1) * TPC],
                                scalar1=bo_sb[:, ot:ot + 1],
                            )
                        nc.sync.dma_start(
                            out=yT2[og * 256:(og + 1) * 256,
                                    b * TPC:(b + 1) * TPC].rearrange(
                                "(ot p) n -> p ot n", p=128),
                            in_=yo[:],
                        )
                    return f

                # slot qs: batch 0 runs chunk qs, batch 1 trails by one chunk
                for qs in range(NQC):
                    for h in range(2):
                        ga = gen_D(0, qs, h)
                        gb = gen_D(1, qs - 1, h) if qs >= 1 else None
                        drive([ga, gb])
                        if pending:
                            pending.pop(0)()
                    for bb, qq in ((0, qs), (1, qs - 1)):
                        if not 0 <= qq < NQC:
                            continue
                        emit_bcnorm(bb, qq, 0)
                        pending.append(
                            (lambda b2=bb, q2=qq: (
                                emit_bcnorm(b2, q2, 1),
                                emit_a2a(b2) if q2 == NQC - 1 else None)))
                # batch 0's A2A fires here, before batch 1's final chunk
                while pending:
                    pending.pop(0)()
                for h in range(2):
                    drive([gen_D(1, NQC - 1, h)])
                emit_bcnorm(1, NQC - 1, 0)
                emit_bcnorm(1, NQC - 1, 1)
                emit_a2a(1)
                # batch 0's out-proj overlaps batch 1's A2A flight
                for og in range(KT_D // 2):
                    u_outproj(0, og)()
                for og in range(KT_D // 2):
                    u_outproj(1, og)()

    _split_waits(nc)
    return nc


def _tile_w(w):
    # [D, N] -> [128, KT_D * N]: contraction tile kt on partitions
    w = np.asarray(w)
    n = w.shape[1]
    return np.ascontiguousarray(
        w.reshape(KT_D, 128, n).transpose(1, 0, 2).reshape(128, KT_D * n))


def kernel(x, mask, Wq, bq, Wk, bk, Wv, bv, Wo, bo, trace=False):
    import ml_dtypes
    bf16 = ml_dtypes.bfloat16
    x = np.asarray(x, dtype=np.float32).reshape(T, D)
    xT = np.ascontiguousarray(x.T).astype(bf16)
    Wo_bf = _tile_w(np.asarray(Wo, np.float32)).astype(bf16)
    bo_f = np.ascontiguousarray(
        np.asarray(bo, np.float32).reshape(KT_D, 128).T)
    in_maps = []
    for c in range(NC):
        sl = slice(c * HG, (c + 1) * HG)
        in_maps.append({
            "xT": xT,
            "wq": _tile_w(np.asarray(Wq, np.float32)[:, sl]).astype(bf16),
            "wk": _tile_w(np.asarray(Wk, np.float32)[:, sl]).astype(bf16),
            "wv": _tile_w(np.asarray(Wv, np.float32)[:, sl]).astype(bf16),
            "wo": Wo_bf,
            "bq": np.ascontiguousarray(np.asarray(bq, np.float32)[sl].reshape(HG, 1)),
            "bk": np.ascontiguousarray(np.asarray(bk, np.float32)[sl].reshape(HG, 1)),
            "bv": np.ascontiguousarray(np.asarray(bv, np.float32)[sl].reshape(HG, 1)),
            "bo": bo_f,
        })
    nc = build_module()
    res = run_bass_kernel_spmd(nc, in_maps, core_ids=list(range(NC)), trace=trace)
    out = np.empty((B, S, D), dtype=np.float32)
    for c in range(NC):
        y = res.results[c]["yT2"]  # [D, B*TPC]
        for b in range(B):
            out[b, c * TPC:(c + 1) * TPC, :] = y[:, b * TPC:(b + 1) * TPC].T
    if trace:
        kernel.last_results = res
    return out.reshape(B, S, D)


# revision 39
# speedup vs baseline: 1.0479x; 1.0479x over previous
"""Multi-head causal self-attention (B=2, S=2048, D=1024, H=16) on 8 TRN2 cores.

Sharding: head-parallel for QKV+attention (core c owns heads {2c, 2c+1}),
token-parallel for the output projection (core c owns tokens
[256c, 256c+256) of each batch), bridged by a per-batch AllToAll of the
normalized context — 8x less wire traffic than gathering or
reduce-scattering partial outputs, since nothing is replicated.

The PE clock gate (HAM) re-throttles to 1.2GHz whenever any 3.4us window
has an idle gap; the attention kg chain (PE scores -> ACT exp -> DVE mask
-> PE ctx) alone always has such gaps. So the emission engine interleaves
dependency-free "filler" matmuls into the PE queue: batch 1's QKV
projection + V-transposes fill batch 0's attention; batch 0's output
projection fills batch 1's attention. Each kg's ctx matmuls are emitted
one kg late (behind the next kg's scores) and bc/normalize consumers one
h-slot late, so the PE queue never waits on the exp or reciprocal chains.

Per core (matmul operands bf16; PSUM accumulation f32):
  stage B: Q^T/K^T/V^T = (x @ W{q,k,v}[:, c-slice] + b)^T   [128, 4096]
  stage C: V^T -> V_aug [tok, 65] tiles (col 64 = ones -> l row)
  stage D: scores^T = K^T.T @ Q^T tiles (PE), exp (ACT, scale=1/8,
           diagonal blocks packed so no masked column is exp'd),
           post-exp multiplicative causal mask (DVE), ctx^T accum (PE)
  stage E: l row -> r = exp(-ln(l)) (ACT, same table as exp), PE
           outer-product broadcast, normalize -> bf16
  stage F: per batch: AllToAll ctx chunks, then out = Wo^T-tiles @
           ctx_full + bo for this core's tokens, full Wo

Host: x pre-transposed bf16; weights host-pre-tiled to [p, kt, n] so all
loads are contiguous; output reassembled token-wise from yT2.
"""

import sys

for p in ("/opt/trn_rl_repo", "/root/.axon_site/_ro/trn_rl_repo"):
    if p not in sys.path:
        sys.path.insert(0, p)

from collections import deque

import numpy as np

import bass_rust
import concourse.bass as bass
import concourse.mybir as mybir
from concourse.bass_utils import run_bass_kernel_spmd
from concourse.masks import make_identity
from concourse.tile import TileContext

B, S, D = 2, 2048, 1024
H, DH = 16, 64
T = B * S              # 4096 tokens
NC = 8                 # cores
HG = D // NC           # 128 qkv dims per core (2 heads)
KT_D = D // 128        # 8 contraction tiles over d_model
QC = 512               # q-chunk width
NQC = S // QC          # 4 q-chunks per batch
TPC = S // NC          # 256 tokens per core per batch (out-proj sharding)
INV_SCALE = 1.0 / float(np.sqrt(DH))  # 1/8
F32 = mybir.dt.float32
F32R = mybir.dt.float32r
BF16 = mybir.dt.bfloat16


def _split_waits(nc, max_waits=1):
    """This walrus build accepts one sync-wait per instruction; Tile sometimes
    emits more. Split extras into preceding NoOps on the same engine."""
    n = 0
    for f in nc.m.functions:
        for bb in f.blocks:
            out = []
            for inst in bb.instructions:
                si = getattr(inst, "sync_info", None)
                if si is not None and si.on_wait and len(si.on_wait) > max_waits:
                    waits = list(si.on_wait)
                    head, rest = waits[:-max_waits], waits[-max_waits:]
                    k = 0
                    while head:
                        chunk, head = head[:max_waits], head[max_waits:]
                        out.append(mybir.InstNoOp(
                            name=f"{inst.name}-wsplit-{k}", ins=[], outs=[],
                            engine=inst.engine,
                            sync_info=bass_rust.SyncInfo(on_wait=chunk, on_update=[]),
                        ))
                        k += 1
                    si.on_wait = rest
                    n += 1
                out.append(inst)
            bb.instructions = out
    return n


def build_module():
    nc = bass.Bass()

    # weights arrive host-pre-tiled ([p, kt, n] flattened) so the loads are
    # fully contiguous DMAs instead of 256B-descriptor gather patterns
    xT = nc.dram_tensor("xT", [D, T], BF16, kind="ExternalInput")
    wq = nc.dram_tensor("wq", [128, KT_D * HG], BF16, kind="ExternalInput")
    wk = nc.dram_tensor("wk", [128, KT_D * HG], BF16, kind="ExternalInput")
    wv = nc.dram_tensor("wv", [128, KT_D * HG], BF16, kind="ExternalInput")
    wo = nc.dram_tensor("wo", [128, KT_D * D], BF16, kind="ExternalInput")
    bq = nc.dram_tensor("bq", [HG, 1], F32, kind="ExternalInput")
    bk = nc.dram_tensor("bk", [HG, 1], F32, kind="ExternalInput")
    bv = nc.dram_tensor("bv", [HG, 1], F32, kind="ExternalInput")
    bo = nc.dram_tensor("bo", [128, KT_D], F32, kind="ExternalInput")
    # output: this core's TPC tokens of each batch, all D dims
    yT2 = nc.dram_tensor("yT2", [D, B * TPC], F32, kind="ExternalOutput")

    # AllToAll buffers: [token-group/src-rank, 128, TPC]
    a2a_in = [nc.dram_tensor(f"a2i{b}", [NC, HG, TPC], BF16) for b in range(B)]
    a2a_out = [nc.dram_tensor(f"a2o{b}", [NC, HG, TPC], BF16) for b in range(B)]

    with TileContext(nc) as tc:
        with tc.tile_pool(name="persist", bufs=1) as pp:
            w_sb = {}
            for name, dram in (("wq", wq), ("wk", wk), ("wv", wv)):
                t = pp.tile([128, KT_D, HG], BF16, name=f"{name}_sb", tag=f"{name}_sb")
                nc.sync.dma_start(out=t[:], in_=dram[:].rearrange("p (kt n) -> p kt n", n=HG))
                w_sb[name] = t
            # gpsimd queue: the 2MB Wo load must not head-of-line block the
            # first xt loads on the sync queue
            wo_sb = pp.tile([128, KT_D, D], BF16, name="wo_sb", tag="wo_sb")
            nc.gpsimd.dma_start(out=wo_sb[:], in_=wo[:].rearrange("p (kt n) -> p kt n", n=D))
            b_sb = {}
            for name, dram in (("bq", bq), ("bk", bk), ("bv", bv)):
                t = pp.tile([HG, 1], F32, name=f"{name}_sb", tag=f"{name}_sb")
                nc.sync.dma_start(out=t[:], in_=dram[:])
                b_sb[name] = t
            bo_sb = pp.tile([128, KT_D], F32, name="bo_sb", tag="bo_sb")
            nc.gpsimd.dma_start(out=bo_sb[:], in_=bo[:])

            ident_f = pp.tile([128, 128], F32, name="ident_f", tag="ident_f")
            make_identity(nc, ident_f[:])
            ident = pp.tile([128, 128], BF16, name="ident", tag="ident")
            nc.vector.tensor_copy(ident[:], ident_f[:])
            # multiplicative causal mask for a diagonal 128x128 tile of
            # scores^T: keep [r, c] where r <= c (k <= q)
            tri_f = pp.tile([128, 128], F32, name="tri_f", tag="tri_f")
            nc.gpsimd.memset(tri_f[:], 1.0)
            nc.gpsimd.affine_select(
                out=tri_f[:], in_=tri_f[:],
                compare_op=mybir.AluOpType.is_ge, fill=0.0,
                base=0, pattern=[[1, 128]], channel_multiplier=-1,
            )
            tri01 = pp.tile([128, 128], BF16, name="tri01", tag="tri01")
            nc.vector.tensor_copy(tri01[:], tri_f[:])
            # ones row at partition 64 (base partition of the l row)
            ones_f = pp.tile([65, DH], F32, name="ones_f", tag="ones_f")
            nc.vector.memset(ones_f[:], 1.0)
            ones_r = pp.tile([65, DH], F32R, name="ones_r", tag="ones_r")
            nc.vector.tensor_copy(ones_r[:], ones_f[:])
            ones128 = pp.tile([128, B * 2 * (S // 128)], F32, name="ones128",
                              tag="ones128")
            nc.vector.memset(ones128[:], 1.0)

            qkvT = {}
            for name in ("qT", "kT", "vT"):
                qkvT[name] = [pp.tile([128, S], BF16, name=f"{name}{b}", tag=f"{name}{b}")
                              for b in range(B)]

            vaug = pp.tile([128, B * 2, S // 128, DH + 1], BF16, name="vaug", tag="vaug")
            nc.vector.tensor_copy(vaug[:, :, :, DH:DH + 1], ones128[:, :])
            # [65 used partitions, pair, q]; row 64 = l
            ctxu = pp.tile([128, B * 2, S], F32, name="ctxu", tag="ctxu")

            def emit_C(b, h, g, pst):
                # V^T -> V_aug transposes for 8 ktiles; pst: [128, >=512] bf16
                # PSUM region
                pr = b * 2 + h
                for j in range(8):
                    kt = g * 8 + j
                    nc.tensor.transpose(
                        out=pst[:, j * DH:(j + 1) * DH],
                        in_=qkvT["vT"][b][h * DH:(h + 1) * DH,
                                          kt * 128:(kt + 1) * 128],
                        identity=ident[h * DH:(h + 1) * DH,
                                       h * DH:(h + 1) * DH],
                    )
                nc.vector.tensor_copy(
                    vaug[:, pr, g * 8:(g + 1) * 8, 0:DH],
                    pst[:, 0:512],
                )

            # ---------------- stage B+C for batch 0 ----------------
            with (
                tc.tile_pool(name="xt0_pool", bufs=3) as xt0_pool,
                tc.tile_pool(name="psB", bufs=1, space="PSUM") as psB_pool,
                tc.tile_pool(name="psT", bufs=2, space="PSUM") as psT_pool,
            ):
                for tq in range(2):
                    t0 = tq * 1024
                    ps = [psB_pool.tile([128, 512], F32, name=f"psB{i}",
                                        tag=f"psB{i}") for i in range(6)]
                    for kt in range(KT_D):
                        xt = xt0_pool.tile([128, 1024], BF16, name="xt", tag="xt")
                        nc.sync.dma_start(
                            out=xt[:],
                            in_=xT[kt * 128:(kt + 1) * 128, t0:t0 + 1024])
                        for pi, wname in enumerate(("wq", "wk", "wv")):
                            for nch in range(2):
                                nc.tensor.matmul(
                                    ps[pi * 2 + nch][:],
                                    w_sb[wname][:, kt, :],
                                    xt[:, nch * 512:(nch + 1) * 512],
                                    start=(kt == 0), stop=(kt == KT_D - 1),
                                )
                    for pi, (dname, bname) in enumerate(
                            (("qT", "bq"), ("kT", "bk"), ("vT", "bv"))):
                        for nch in range(2):
                            nc.vector.tensor_scalar_add(
                                out=qkvT[dname][0][:, t0 + nch * 512:
                                                   t0 + (nch + 1) * 512],
                                in0=ps[pi * 2 + nch][:],
                                scalar1=b_sb[bname][:, 0:1],
                            )
                for h in range(2):
                    for g in range(2):
                        pst = psT_pool.tile([128, 512], BF16, name="pst", tag="pst")
                        emit_C(0, h, g, pst[:])

            # ------- stages D-F + interleaved B(b1)/C(b1)/F'(b0) -------
            with (
                tc.tile_pool(name="psS", bufs=2, space="PSUM") as psS_pool,
                tc.tile_pool(name="psC", bufs=1, space="PSUM") as psC_pool,
                tc.tile_pool(name="mps", bufs=1, space="PSUM") as mps_pool,
                tc.tile_pool(name="psBI", bufs=1, space="PSUM") as psBI_pool,
                tc.tile_pool(name="xt_pool", bufs=16) as xt_pool,
                tc.tile_pool(name="exp_pool", bufs=4) as exp_pool,
                tc.tile_pool(name="rpool", bufs=4) as rpool,
                tc.tile_pool(name="cn_pool", bufs=4) as cn_pool,
                tc.tile_pool(name="cf_pool", bufs=2) as cf_pool,
                tc.tile_pool(name="yo_pool", bufs=2) as yo_pool,
            ):
                r_tiles = {}
                cn_tiles = {}
                ctxf_tiles = {}
                filler = deque()   # dependency-free PE work units
                pending = []       # deferred bc/normalize slots

                # ---- B(b1) interleave units ----
                # 2-chunk groups share 8 live [128,1024] xt tiles; each
                # 512-chunk runs 3 passes (q, k, v) over 2 accumulator banks.
                xt1_tiles = {}     # tqgroup -> list of 8 tiles

                def u_load(tqg, half):
                    def f():
                        tiles = xt1_tiles.setdefault(tqg, [])
                        for kt in range(half * 4, half * 4 + 4):
                            xt = xt_pool.tile([128, 1024], BF16, name="xt1",
                                              tag="xt1")
                            nc.sync.dma_start(
                                out=xt[:],
                                in_=xT[kt * 128:(kt + 1) * 128,
                                       S + tqg * 1024: S + tqg * 1024 + 1024])
                            tiles.append(xt)
                    return f

                def u_pass(tqg, ch, wname, bname, acc_tag):
                    # one full projection pass for 512-token chunk ch of
                    # group tqg: 8 accumulate MMs + bias drain
                    def f():
                        acc = psBI_pool.tile([128, 512], F32, name=acc_tag,
                                             tag=acc_tag)
                        tiles = xt1_tiles[tqg]
                        for kt in range(KT_D):
                            nc.tensor.matmul(
                                acc[:],
                                w_sb[wname][:, kt, :],
                                tiles[kt][:, ch * 512:(ch + 1) * 512],
                                start=(kt == 0), stop=(kt == KT_D - 1),
                            )
                        t0 = tqg * 1024 + ch * 512
                        nc.vector.tensor_scalar_add(
                            out=qkvT[wname.replace("w", "") + "T"][1][:, t0:t0 + 512],
                            in0=acc[:],
                            scalar1=b_sb[bname][:, 0:1],
                        )
                    return f

                def u_c1(h, g):
                    def f():
                        # transpose scratch borrowed from a psS tile (bitcast
                        # f32 bank region to bf16)
                        ps_t = psS_pool.tile([128, 1024], F32, name="ps_s",
                                             tag="ps_s")
                        emit_C(1, h, g, ps_t[:].bitcast(BF16))
                    return f

                for tqg in range(2):
                    filler.append(u_load(tqg, 0))
                    filler.append(u_load(tqg, 1))
                    for ch in range(2):
                        for wname, bname, acc in (("wq", "bq", "accA"),
                                                  ("wk", "bk", "accB"),
                                                  ("wv", "bv", "accA")):
                            filler.append(u_pass(tqg, ch, wname, bname, acc))
                    if tqg == 1:
                        for h in range(2):
                            for g in range(2):
                                filler.append(u_c1(h, g))

                def emit_D(b, qc, h, fill_every=2):
                    q0 = qc * QC
                    n_kt = q0 // 128 + 4
                    pr = b * 2 + h
                    qT_h = qkvT["qT"][b][h * DH:(h + 1) * DH, :]
                    kT_h = qkvT["kT"][b][h * DH:(h + 1) * DH, :]
                    ps_ctx = psC_pool.tile([128, QC], F32, name="ps_ctx",
                                           tag="ps_ctx")

                    def ctx_mms(ka, kb, offa, offb, ex):
                        nc.tensor.matmul(
                            ps_ctx[0:DH + 1, offa:512],
                            vaug[:, pr, ka, :],
                            ex[:, offa:512],
                            start=(ka == 0), stop=False,
                            skip_group_check=True,
                        )
                        nc.tensor.matmul(
                            ps_ctx[0:DH + 1, offb:512],
                            vaug[:, pr, kb, :],
                            ex[:, 512:1024 - offb],
                            start=False, stop=(kb == n_kt - 1),
                            skip_group_check=True,
                        )

                    prev_ctx = None
                    for kg in range(n_kt // 2):
                        ka, kb = 2 * kg, 2 * kg + 1
                        offa = max(0, ka * 128 - q0)
                        offb = max(0, kb * 128 - q0)
                        ps_s = psS_pool.tile([128, 1024], F32, name="ps_s",
                                             tag="ps_s")
                        # kb's block is packed at column 512 (width 512-offb)
                        # so the exp range [offa:1024-offb] has no dead gap
                        nc.tensor.matmul(
                            ps_s[:, offa:512],
                            kT_h[:, ka * 128:(ka + 1) * 128],
                            qT_h[:, q0 + offa:q0 + 512],
                            start=True, stop=True,
                        )
                        nc.tensor.matmul(
                            ps_s[:, 512:1024 - offb],
                            kT_h[:, kb * 128:(kb + 1) * 128],
                            qT_h[:, q0 + offb:q0 + 512],
                            start=True, stop=True,
                        )
                        # the PREVIOUS kg's ctx matmuls go behind this kg's
                        # scores: their exp input is ready by now, so the PE
                        # queue never waits on the exp chain
                        if prev_ctx is not None:
                            prev_ctx()
                        ex = exp_pool.tile([128, 1024], BF16, name="ex", tag="ex")
                        nc.scalar.activation(
                            out=ex[:, offa:1024 - offb],
                            in_=ps_s[:, offa:1024 - offb],
                            func=mybir.ActivationFunctionType.Exp,
                            scale=INV_SCALE,
                        )
                        # causal mask: multiplicative 0/1 on the diagonal
                        # tiles, applied AFTER exp (off the PE->ACT path)
                        if ka * 128 >= q0:
                            nc.vector.tensor_mul(
                                out=ex[:, offa:offa + 128],
                                in0=ex[:, offa:offa + 128],
                                in1=tri01[:],
                            )
                        if kb * 128 >= q0:
                            nc.vector.tensor_mul(
                                out=ex[:, 512:640],
                                in0=ex[:, 512:640],
                                in1=tri01[:],
                            )
                        prev_ctx = (lambda a=ka, b2=kb, oa=offa, ob=offb,
                                    e=ex: ctx_mms(a, b2, oa, ob, e))
                        if filler and kg % fill_every == fill_every - 1:
                            filler.popleft()()
                    prev_ctx()
                    nc.vector.tensor_copy(
                        ctxu[0:DH + 1, pr, q0:q0 + 512],
                        ps_ctx[0:DH + 1, :],
                    )
                    # r = 1/l = exp(-ln(l)): ln/exp share the attention exp's
                    # ACT table; the exp writes f32r (a rounding op, so the
                    # f32r matmul consumer passes BIR verification)
                    ln_f = rpool.tile([65, QC], F32, name="ln_f", tag="ln_f")
                    nc.scalar.activation(
                        out=ln_f[64:65, :], in_=ctxu[64:65, pr, q0:q0 + QC],
                        func=mybir.ActivationFunctionType.Ln)
                    r_t = rpool.tile([65, QC], F32R, name="r_t", tag="r_t")
                    nc.scalar.activation(
                        out=r_t[64:65, :], in_=ln_f[64:65, :],
                        func=mybir.ActivationFunctionType.Exp, scale=-1.0)
                    r_tiles[(b, qc, h)] = r_t

                def emit_bcnorm(b, qc, h):
                    # deferred one h-slot: the PE bc matmul's reciprocal input
                    # is long since ready, so the PE queue never stalls here
                    q0 = qc * QC
                    pr = b * 2 + h
                    if h == 0:
                        cn_tiles[(b, qc)] = cn_pool.tile(
                            [128, QC], BF16, name="cn", tag="cn")
                    cn = cn_tiles[(b, qc)]
                    r_t = r_tiles.pop((b, qc, h))
                    bc = mps_pool.tile([128, QC], F32, name="bc", tag="mps")
                    nc.tensor.matmul(
                        bc[0:DH, :],
                        ones_r[64:65, 0:DH],
                        r_t[64:65, :],
                        start=True, stop=True,
                    )
                    nc.vector.tensor_mul(
                        out=cn[h * DH:(h + 1) * DH, :],
                        in0=ctxu[0:DH, pr, q0:q0 + QC],
                        in1=bc[0:DH, :],
                    )
                    if h == 1:
                        # chunk complete: ship to the A2A input buffer
                        # (token groups 2qc, 2qc+1). Sync queue — the gpsimd
                        # queue blocks on in-flight collectives.
                        nc.sync.dma_start(
                            out=a2a_in[b][:].rearrange("g p n -> p g n")[
                                :, 2 * qc:2 * qc + 2, :],
                            in_=cn[:].rearrange("p (g n) -> p g n", g=2),
                        )
                        del cn_tiles[(b, qc)]

                def emit_a2a(b):
                    nc.gpsimd.collective_compute(
                        "AllToAll",
                        mybir.AluOpType.bypass,
                        ins=[a2a_in[b][:]],
                        outs=[a2a_out[b][:]],
                        replica_groups=[list(range(NC))],
                    )

                def emit_ctxf_load(b):
                    ctxf = cf_pool.tile([128, KT_D, TPC], BF16, name="ctxf",
                                        tag="ctxf")
                    nc.gpsimd.dma_start(
                        out=ctxf[:],
                        in_=a2a_out[b][:].rearrange("kt p n -> p kt n"))
                    ctxf_tiles[b] = ctxf

                def u_outproj(b, og):
                    # two out-dim tiles of batch b's token-sharded output
                    # projection (shares the mps bank with bc)
                    def f():
                        ctxf = ctxf_tiles[b]
                        ps_o = mps_pool.tile([128, QC], F32, name="ps_o",
                                             tag="mps")
                        for sub in range(2):
                            ot = og * 2 + sub
                            for kt in range(KT_D):
                                nc.tensor.matmul(
                                    ps_o[:, sub * TPC:(sub + 1) * TPC],
                                    wo_sb[:, kt, ot * 128:(ot + 1) * 128],
                                    ctxf[:, kt, :],
                                    start=(kt == 0), stop=(kt == KT_D - 1),
                                    skip_group_check=True,
                                )
                        yo = yo_pool.tile([128, 2, TPC], F32, name="yo", tag="yo")
                        for sub in range(2):
                            ot = og * 2 + sub
                            nc.vector.tensor_scalar_add(
                                out=yo[:, sub, :],
                                in0=ps_o[:, sub * TPC:(sub + 1) * TPC],
                                scalar1=bo_sb[:, ot:ot + 1],
                            )
                        nc.sync.dma_start(
                            out=yT2[og * 256:(og + 1) * 256,
                                    b * TPC:(b + 1) * TPC].rearrange(
                                "(ot p) n -> p ot n", p=128),
                            in_=yo[:],
                        )
                    return f

                for b in range(B):
                    for qc in range(NQC):
                        # batch 1's attention interleaves the (sparser)
                        # out-proj units less often
                        fe = 2 if b == 0 else 5
                        emit_D(b, qc, 0, fill_every=fe)
                        if pending:
                            pending.pop(0)()
                        emit_D(b, qc, 1, fill_every=fe)
                        emit_bcnorm(b, qc, 0)
                        pending.append(
                            (lambda bb=b, qq=qc: (
                                emit_bcnorm(bb, qq, 1),
                                emit_a2a(bb) if qq == NQC - 1 else None)))
                    if b == 0:
                        # drain any leftover B(b1)/C(b1) units before D(b1)
                        # needs their outputs
                        while filler:
                            filler.popleft()()
                        # b0's bc/norm tail + A2A, then queue its out-proj
                        # units as D(b1) filler
                        while pending:
                            pending.pop(0)()
                        emit_ctxf_load(0)
                        for og in range(KT_D // 2):
                            filler.append(u_outproj(0, og))
                while pending:
                    pending.pop(0)()
                while filler:
                    filler.popleft()()
                emit_ctxf_load(1)
                for og in range(KT_D // 2):
                    u_outproj(1, og)()

    _split_waits(nc)
    return nc


def _tile_w(w):
    # [D, N] -> [128, KT_D * N]: contraction tile kt on partitions
    w = np.asarray(w)
    n = w.shape[1]
    return np.ascontiguousarray(
        w.reshape(KT_D, 128, n).transpose(1, 0, 2).reshape(128, KT_D * n))


def kernel(x, mask, Wq, bq, Wk, bk, Wv, bv, Wo, bo, trace=False):
    import ml_dtypes
    bf16 = ml_dtypes.bfloat16
    x = np.asarray(x, dtype=np.float32).reshape(T, D)
    xT = np.ascontiguousarray(x.T).astype(bf16)
    Wo_bf = _tile_w(np.asarray(Wo, np.float32)).astype(bf16)
    bo_f = np.ascontiguousarray(
        np.asarray(bo, np.float32).reshape(KT_D, 128).T)
    in_maps = []
    for c in range(NC):
        sl = slice(c * HG, (c + 1) * HG)
        in_maps.append({
            "xT": xT,
            "wq": _tile_w(np.asarray(Wq, np.float32)[:, sl]).astype(bf16),
            "wk": _tile_w(np.asarray(Wk, np.float32)[:, sl]).astype(bf16),
            "wv": _tile_w(np.asarray(Wv, np.float32)[:, sl]).astype(bf16),
            "wo": Wo_bf,
            "bq": np.ascontiguousarray(np.asarray(bq, np.float32)[sl].reshape(HG, 1)),
            "bk": np.ascontiguousarray(np.asarray(bk, np.float32)[sl].reshape(HG, 1)),
            "bv": np.ascontiguousarray(np.asarray(bv, np.float32)[sl].reshape(HG, 1)),
            "bo": bo_f,
        })
    nc = build_module()
    res = run_bass_kernel_spmd(nc, in_maps, core_ids=list(range(NC)), trace=trace)
    out = np.empty((B, S, D), dtype=np.float32)
    for c in range(NC):
        y = res.results[c]["yT2"]  # [D, B*TPC]
        for b in range(B):
            out[b, c * TPC:(c + 1) * TPC, :] = y[:, b * TPC:(b + 1) * TPC].T
    if trace:
        kernel.last_results = res
    return out.reshape(B, S, D)
